# revision 47
# baseline (speedup 1.0000x reference)
"""Trainium2 Bass kernel for a scalar-input GRU (B=512, T=128, H=512) + ReLU/Linear head.

Strategy: data-parallel over batch across 8 NeuronCores (64 rows each).

Layout: "transposed" / weights-stationary. All per-step tensors live as
[hidden-dim on partitions, batch on free]: gate pre-activations are computed as
  ghT[j, b] = sum_k w_hh[j, k] * h[b, k]
with the w_hh block as the PE stationary operand ([K=128, M=128], full array)
and hT chunks as the moving operand (N=64 rows streamed per matmul). h_new is
produced directly in this layout, so it IS the next step's moving operand —
no transposes anywhere in the recurrence.

Precision/speed: the h-recurrence matmuls run in fp8e4m3 with the DoubleRow
perf mode (2 K-tiles of 128 per instruction at 0.5 cycles/row); weights are
pre-scaled by Sw=8 and h by Sh=8 so all fp8 values sit in the normal range.
The scalar-input terms (x_t * w_ih + biases, pre-scaled by S=64) are injected
by small K=2 bf16 "augmented" matmuls into the same PSUM accumulation groups.
The S=64 scaling is undone for free via the ACT engine's scale operand on the
sigmoid/tanh. Gate algebra runs in bf16 on DVE/Pool, with the per-engine
emission order chosen by sweep (the engine sequencers are FIFOs, so program
order IS the schedule; the step period is bound by the cross-step serial
chain sigmoid -> r*ghn -> +gx_n -> tanh -> h-update -> fp8 cast -> matmuls).
Verified: rel err 7.9e-3 (tolerance 2e-2), 474339 ns vs 873366 ns baseline.

All weight/layout prep (transposition, quantization, scaling, interleaved
x/ones moving layout) happens host-side in numpy; the device program just DMAs
ready-made tensors.
"""

import sys

sys.path.insert(0, "/opt/trn_rl_repo")

import ml_dtypes
import numpy as np

import concourse.bacc as bacc
import concourse.bass as bass
import concourse.mybir as mybir
from concourse.bass_utils import run_bass_kernel_spmd
import concourse.tile as tile

N_CORES = 8
B_FULL, T_FULL, H = 512, 128, 512
B = B_FULL // N_CORES  # 64 batch rows per core
G3 = 3 * H
F32 = mybir.dt.float32
BF16 = mybir.dt.bfloat16
FP8 = mybir.dt.float8e4
AF = mybir.ActivationFunctionType
DR = mybir.MatmulPerfMode.DoubleRow

NP_BF16 = ml_dtypes.bfloat16
NP_FP8 = ml_dtypes.float8_e4m3

SW = 8.0  # weight scale
SH = 8.0  # h scale
S = SW * SH  # combined pre-activation scale

# emission order of the per-step elementwise ops (per-engine program order is
# the schedule). Overridable for sweeps via BASS_SCHED env.
import os as _os

_SCHEDS = {
    "a": ["sr", "sz", "m0", "t20", "m1", "t21", "th0", "th1",
          "zc0", "zc1", "u0", "u1", "v0", "h80", "v1", "h81", "hb0", "hb1"],
    "b": ["sr", "sz", "m1", "t21", "m0", "t20", "th1", "th0",
          "zc1", "zc0", "u1", "u0", "v1", "h81", "v0", "h80", "hb1", "hb0"],
    "c": ["sr", "sz", "m0", "m1", "t20", "t21", "th0", "th1",
          "zc0", "zc1", "u0", "u1", "v0", "h80", "v1", "h81", "hb0", "hb1"],
    "d": ["sr", "sz", "m0", "t20", "m1", "t21", "th0", "th1",
          "zc0", "zc1", "u0", "u1", "v0", "v1", "h80p", "h81p", "hb0", "hb1"],
    "e": ["sr0", "sr1", "sz", "m0", "t20", "m1", "t21", "th0", "th1",
          "zc0", "zc1", "u0", "u1", "v0", "h80", "v1", "h81", "hb0", "hb1"],
    "f": ["sr", "sz", "m0", "t20", "m1", "t21", "th0", "th1",
          "zc0", "zc1", "u0", "u1", "v0", "h80p", "v1", "h81", "hb0", "hb1"],
    "g": ["sr1", "sr0", "sz", "m1", "t21", "m0", "t20", "th1", "th0",
          "zc1", "zc0", "u1", "u0", "v1", "h81", "v0", "h80", "hb1", "hb0"],
    "h": ["sr", "sz", "m0", "t20", "m1", "t21", "th0", "th1",
          "zc0", "zc1", "u0d", "u1d", "v0", "h80", "v1", "h81", "hb0", "hb1"],
    # 3-op tail (d,e,h8) without zc/u: fewer total ops
    "i": ["sr", "sz", "m0", "t20", "m1", "t21", "th0", "th1",
          "d0", "e0", "h8s0", "d1", "e1", "h8s1", "n80", "hbn0", "n81", "hbn1"],
    "j": ["sr", "sz", "m1", "t21", "m0", "t20", "th1", "th0",
          "d1", "e1", "h8s1", "d0", "e0", "h8s0", "n81", "hbn1", "n80", "hbn0"],
    "k": ["gfirst", "sr", "sz", "m0", "t20", "m1", "t21", "th0", "th1",
          "zc0", "zc1", "u0", "u1", "v0", "h80", "v1", "h81", "hb0", "hb1"],
    "l": ["gfirst", "sr", "sz", "m0", "t20", "m1", "t21", "th0", "th1",
          "d0", "e0", "h8s0", "d1", "e1", "h8s1", "n80", "hbn0", "n81", "hbn1"],
    # full-width single ops: fewer FIFO slots / crossings, no half interference
    "m": ["sr", "sz", "mf", "t2f", "thf", "zcf", "uf", "vf", "h8f", "hbf"],
    "n": ["sr", "sz", "mf", "t2f", "thf", "df", "ef", "h8sf", "n8f", "hbnf"],
    "o": ["gfirst", "sr", "sz", "mf", "t2f", "thf", "zcf", "uf", "vf", "h8f", "hbf"],
    "p": ["sr", "sz", "mf", "t2f", "thf", "zcf", "uf", "vf", "h8f", "hbfd"],
    # the measured-best config: split sigmoids, zc/u full-width on DVE
    "q": ["sr0", "sr1", "sz", "m0", "t20", "m1", "t21", "th0", "th1",
          "zcfd", "ufd", "v0", "h80", "v1", "h81", "hb0", "hb1"],
    "r": ["sr0", "sr1", "sz", "m1", "t21", "m0", "t20", "th1", "th0",
          "zcfd", "ufd", "v1", "h81", "v0", "h80", "hb1", "hb0"],
    "s": ["sr", "sz", "m0", "t20", "m1", "t21", "th0", "th1",
          "zcfd", "ufd", "v0", "h80", "v1", "h81", "hb0", "hb1"],
    "u": ["gfirst", "sr0", "sr1", "sz", "m0", "t20", "m1", "t21", "th0", "th1",
          "zcfd", "ufd", "v0", "h80", "v1", "h81", "hb0", "hb1"],
    "t": ["sr", "sz", "m0", "t20", "m1", "t21", "th0", "th1",
          "zcf", "ufd", "v0", "h80", "v1", "h81", "hb0", "hb1"],
    "w": ["gfirst", "sr", "sz", "m0", "t20", "m1", "t21", "th0", "th1",
          "zcfd", "ufd", "v0", "h80", "v1", "h81", "hb0", "hb1"],
    "x": ["sr", "sz", "m0", "t20", "m1", "t21", "th0", "th1",
          "zcfd", "ufd", "v0", "h80", "v1", "h81p", "hb0", "hb1"],
    "y": ["sr", "sz", "m1", "t21", "m0", "t20", "th1", "th0",
          "zcfd", "ufd", "v1", "h81", "v0", "h80", "hb1", "hb0"],
    "s2": ["sr", "sz", "m0", "zcfd", "t20", "m1", "ufd", "t21", "th0", "th1",
           "v0", "h80", "v1", "h81", "hb0", "hb1"],
    "s3": ["sr", "sz", "m0", "t20", "m1", "t21", "th0", "th1",
           "zcfd", "ufd", "v0", "v1", "h80", "h81", "hb0", "hb1"],
    "s5": ["sr", "sz", "szc", "m0", "t20", "m1", "t21", "th0", "th1",
           "ufd", "v0", "h80", "v1", "h81", "hb0", "hb1"],
    "s7": ["sr", "sz", "mf", "t2f", "th0F", "th1F",
           "zcfd", "ufd", "v0", "h80", "v1", "h81", "hb0", "hb1"],
    "s7a": ["sr", "sz", "szc", "mf", "t2f", "th0F", "th1F",
            "ufd", "v0", "h80", "v1", "h81", "hb0", "hb1"],
    "s7b": ["sr", "sz", "mf", "t2f", "th0F", "th1F",
            "zcfd", "ufd", "v0", "v1", "h80", "h81", "hb0", "hb1"],
    "s7d": ["gfirst", "sr", "sz", "mf", "t2f", "th0F", "th1F",
            "zcfd", "ufd", "v0", "h80", "v1", "h81", "hb0", "hb1"],
    "s7e": ["sr", "sz", "mf", "zcfd", "t2f", "ufd", "th0F", "th1F",
            "v0", "h80", "v1", "h81", "hb0", "hb1"],
    "s8": ["sr", "sz", "mf", "t20g", "t21g", "th0", "th1",
           "zcfd", "ufd", "v0", "h80", "v1", "h81", "hb0", "hb1"],
    "s8b": ["sr", "sz", "mf", "t20g", "t21g", "th0", "th1",
            "zcfd", "ufd", "v0", "v1", "h80", "h81", "hb0", "hb1"],
    "s9": ["sr", "sz", "mf", "t21g", "t20g", "th1", "th0",
           "zcfd", "ufd", "v1", "h81", "v0", "h80", "hb1", "hb0"],
    "s9b": ["sr", "sz", "mf", "t21g", "th1", "t20g", "th0",
            "zcfd", "ufd", "v1", "h81", "v0", "h80", "hb1", "hb0"],
    "s9c": ["sr", "sz", "zcfd", "mf", "t21g", "t20g", "th1", "th0",
            "ufd", "v1", "h81", "v0", "h80", "hb1", "hb0"],
    "s11": ["srp", "sz", "mfp", "t20g", "t21g", "th0", "th1",
            "zcfd", "ufd", "v0", "h80", "v1", "h81", "hb0", "hb1"],
    "s12": ["sr", "sz", "mf", "zcfd", "t20g", "ufd", "t21g", "th0", "th1",
            "v0", "h80", "v1", "h81", "hb0", "hb1"],
    "s13": ["sr", "sz", "mf", "t20g", "t21g", "th0", "th1",
            "zcfd", "v0", "ufd", "h80", "v1", "h81", "hb0", "hb1"],
    "s14": ["sr", "sz", "mf", "t20g", "t21g", "th0", "th1",
            "zc0", "zc1", "u0", "u1", "v0", "h80", "v1", "h81", "hb0", "hb1"],
    "s15": ["sr", "sz", "mf", "t20g", "t21g", "th0", "th1",
            "zcfd", "ufd", "v0", "v1", "h80p", "h81", "hb0", "hb1"],
    "s16": ["sr", "sz", "mf", "t20g", "t21g", "th0", "th1",
            "zcfd", "ufd", "v1", "h81", "v0", "h80p", "hb0", "hb1"],
}
SCHED = _SCHEDS[_os.environ.get("BASS_SCHED", "s8")]


def build_nc(T: int = T_FULL) -> bass.Bass:
    nc = bacc.Bacc("TRN2", target_bir_lowering=False, debug=False)

    ws_d = nc.dram_tensor("ws", [128, 12 * 4 * 128], FP8, kind="ExternalInput")
    aug_d = nc.dram_tensor("aug", [2, 16 * 128], BF16, kind="ExternalInput")
    xa_d = nc.dram_tensor("xa", [2, T * B], BF16, kind="ExternalInput")
    fcw_d = nc.dram_tensor("fcw", [128, 4], BF16, kind="ExternalInput")
    fcb_d = nc.dram_tensor("fcb", [1, 1], F32, kind="ExternalInput")
    out_d = nc.dram_tensor("out", [B, 1], F32, kind="ExternalOutput")

    with tile.TileContext(nc) as tc:
        _body(tc, T, ws_d, aug_d, xa_d, fcw_d, fcb_d, out_d)
    nc.compile()
    return nc


def _body(tc, T, ws_d, aug_d, xa_d, fcw_d, fcb_d, out_d):
    nc = tc.nc
    with (
        tc.tile_pool(name="const", bufs=1) as cpool,
        tc.tile_pool(name="state", bufs=2) as spool,
        tc.tile_pool(name="work", bufs=3) as wpool,
        tc.tile_pool(name="psrz", bufs=2, space="PSUM") as przpool,
        tc.tile_pool(name="psng", bufs=2, space="PSUM") as pngpool,
    ):
        # ---- load host-prepped constants ----
        WS = cpool.tile([128, 12 * 4 * 128], FP8)
        nc.sync.dma_start(out=WS[:, :], in_=ws_d[:, :])
        AUG = cpool.tile([2, 16 * 128], BF16)
        nc.sync.dma_start(out=AUG[:, :], in_=aug_d[:, :])
        XA = cpool.tile([2, T * B], BF16)
        nc.sync.dma_start(out=XA[:, :], in_=xa_d[:, :])
        FCW = cpool.tile([128, 4], BF16)
        nc.sync.dma_start(out=FCW[:, :], in_=fcw_d[:, :])
        FCB = cpool.tile([1, 1], F32)
        nc.sync.dma_start(out=FCB[:, :], in_=fcb_d[:, :])
        ONES = cpool.tile([1, B], F32)
        nc.gpsimd.memset(ONES[:, :], 1.0)

        # state: h~ = 8*h (bf16 master, pre-scaled) and h8 = fp8(8*h)
        h_bf = spool.tile([128, 4 * B], BF16, tag="h", name="h_init")
        h8 = spool.tile([128, 4 * B], FP8, tag="h8", name="h8_init")
        nc.gpsimd.memset(h_bf[:, :], 0.0)
        nc.gpsimd.memset(h8[:, :], 0.0)

        # slice order in WS / AUG: r0..r3, z0..z3, n0..n3 (s = g*4+c); AUG has
        # 4 extra "psG" slices (x*wi_n + b_ih_n) at s' = 12..15.
        def w_blk(s, p):
            base = (s * 2 + p) * 2 * 128
            return WS[:, base : base + 256].rearrange("p (i m) -> p i m", i=2)

        def aug_blk(s):
            return AUG[0:2, s * 128 : (s + 1) * 128]

        # ---- the recurrence, fully unrolled ----
        for t in range(T):
            psRZ = przpool.tile([128, 512], F32, tag="rz", name=f"psRZ_{t}")
            psN = pngpool.tile([128, 256], F32, tag="n", name=f"psN_{t}")
            psG = pngpool.tile([128, 256], F32, tag="g", name=f"psG_{t}")
            r_ps = (pngpool.tile([128, 256], F32, tag="rp", name=f"rps_{t}")
                    if "srp" in SCHED else None)
            xr = XA[0:2, 64 * t : 64 * t + 64]

            def hmove(p):
                return h8[:, 128 * p : 128 * p + 128].rearrange(
                    "p (i b) -> p i b", i=2
                )

            def slice_mms(ps, col, s):
                nc.tensor.matmul(
                    ps[:, col : col + 64], aug_blk(s), xr, start=True, stop=False
                )
                nc.tensor.matmul(
                    ps[:, col : col + 64], w_blk(s, 0), hmove(0),
                    start=False, stop=False, perf_mode=DR,
                )
                nc.tensor.matmul(
                    ps[:, col : col + 64], w_blk(s, 1), hmove(1),
                    start=False, stop=True, perf_mode=DR,
                )

            r_sb = wpool.tile([128, 256], BF16, tag="r", name=f"r_{t}")
            z_sb = wpool.tile([128, 256], BF16, tag="z", name=f"z_{t}")
            zc_sb = wpool.tile([128, 256], BF16, tag="zc", name=f"zc_{t}")
            n_sb = wpool.tile([128, 256], BF16, tag="n", name=f"n_{t}")
            u_sb = wpool.tile([128, 256], BF16, tag="u", name=f"u_{t}")
            hn_bf = spool.tile([128, 4 * B], BF16, tag="h", name=f"h_{t}")
            hn_8 = spool.tile([128, 4 * B], FP8, tag="h8", name=f"h8_{t}")
            m_sb = [
                wpool.tile([128, 128], BF16, tag="m0", name=f"m0_{t}"),
                wpool.tile([128, 128], BF16, tag="m1", name=f"m1_{t}"),
            ]
            t2_sb = [
                wpool.tile([128, 128], BF16, tag="t20", name=f"t20_{t}"),
                wpool.tile([128, 128], BF16, tag="t21", name=f"t21_{t}"),
            ]
            v_sb = [
                wpool.tile([128, 128], BF16, tag="v0", name=f"v0_{t}"),
                wpool.tile([128, 128], BF16, tag="v1", name=f"v1_{t}"),
            ]
            HH = (slice(0, 128), slice(128, 256))

            # PE: per-slice atomic groups (interleaving groups corrupts PSUM)
            def emit_gaugs():
                for c in range(4):
                    nc.tensor.matmul(psG[:, 64 * c : 64 * c + 64], aug_blk(12 + c),
                                     xr, start=True, stop=True)

            if "gfirst" in SCHED:
                emit_gaugs()
            for c in range(4):
                slice_mms(psRZ, 64 * c, 0 + c)        # r_c
            for c in range(4):
                slice_mms(psRZ, 256 + 64 * c, 4 + c)  # z_c
            for c in range(4):
                slice_mms(psN, 64 * c, 8 + c)         # n_c
            if "gfirst" not in SCHED:
                emit_gaugs()

            # elementwise ops, emitted in the order given by SCHED (the
            # per-engine program order IS the schedule)
            def op_sr(h=None):
                if h is None:
                    nc.scalar.activation(r_sb[:, :], psRZ[:, 0:256], AF.Sigmoid,
                                         scale=1.0 / S)
                else:
                    nc.scalar.activation(r_sb[:, HH[h]], psRZ[:, HH[h]],
                                         AF.Sigmoid, scale=1.0 / S)

            def op_sz():
                nc.scalar.activation(z_sb[:, :], psRZ[:, 256:512], AF.Sigmoid,
                                     scale=1.0 / S)

            def op_th(h):
                nc.scalar.activation(n_sb[:, HH[h]], t2_sb[h][:, :], AF.Tanh,
                                     scale=1.0 / S)

            def op_m(h):
                nc.vector.tensor_mul(m_sb[h][:, :], psN[:, HH[h]], r_sb[:, HH[h]])

            def op_t2(h):
                nc.vector.tensor_add(t2_sb[h][:, :], psG[:, HH[h]], m_sb[h][:, :])

            def op_zc(h, eng):
                # zc8 = 8 - 8z (the *8 of v folded in; exact in bf16)
                eng.tensor_scalar(zc_sb[:, HH[h]], z_sb[:, HH[h]], -SH, SH,
                                  op0=mybir.AluOpType.mult, op1=mybir.AluOpType.add)

            def op_u(h, eng):
                eng.tensor_mul(u_sb[:, HH[h]], z_sb[:, HH[h]], h_bf[:, HH[h]])

            def op_v(h, eng):
                # v = n * zc8 = 8n(1-z): plain TT gets the DVE 2x bf16 mode
                eng.tensor_mul(v_sb[h][:, :], n_sb[:, HH[h]], zc_sb[:, HH[h]])

            def op_h8(h, eng):
                eng.tensor_add(hn_8[:, HH[h]], u_sb[:, HH[h]], v_sb[h][:, :])

            def op_hb(h, eng):
                eng.tensor_add(hn_bf[:, HH[h]], u_sb[:, HH[h]], v_sb[h][:, :])

            # 3-op tail: d = h~ - 8n ; e = z*d ; h8 = 8n + e
            d_sb = [
                wpool.tile([128, 128], BF16, tag="d0", name=f"d0_{t}"),
                wpool.tile([128, 128], BF16, tag="d1", name=f"d1_{t}"),
            ]
            e_sb = [
                wpool.tile([128, 128], BF16, tag="e0", name=f"e0_{t}"),
                wpool.tile([128, 128], BF16, tag="e1", name=f"e1_{t}"),
            ]
            n8_sb = [
                wpool.tile([128, 128], BF16, tag="n80", name=f"n80_{t}"),
                wpool.tile([128, 128], BF16, tag="n81", name=f"n81_{t}"),
            ]

            def op_d(h):
                nc.vector.scalar_tensor_tensor(
                    d_sb[h][:, :], n_sb[:, HH[h]], -SH, h_bf[:, HH[h]],
                    op0=mybir.AluOpType.mult, op1=mybir.AluOpType.add,
                )

            def op_e(h):
                nc.vector.tensor_mul(e_sb[h][:, :], z_sb[:, HH[h]], d_sb[h][:, :])

            def op_h8s(h):
                nc.vector.scalar_tensor_tensor(
                    hn_8[:, HH[h]], n_sb[:, HH[h]], SH, e_sb[h][:, :],
                    op0=mybir.AluOpType.mult, op1=mybir.AluOpType.add,
                )

            def op_n8(h):
                nc.gpsimd.tensor_scalar_mul(n8_sb[h][:, :], n_sb[:, HH[h]], SH)

            def op_hbn(h):
                nc.gpsimd.tensor_add(hn_bf[:, HH[h]], n8_sb[h][:, :], e_sb[h][:, :])

            # full-width ops
            mF = wpool.tile([128, 256], BF16, tag="mF", name=f"mF_{t}")
            t2F = wpool.tile([128, 256], BF16, tag="t2F", name=f"t2F_{t}")
            vF = wpool.tile([128, 256], BF16, tag="vF", name=f"vF_{t}")
            dF = wpool.tile([128, 256], BF16, tag="dF", name=f"dF_{t}")
            eF = wpool.tile([128, 256], BF16, tag="eF", name=f"eF_{t}")
            n8F = wpool.tile([128, 256], BF16, tag="n8F", name=f"n8F_{t}")

            OPS = {
                "gfirst": lambda: None,
                "d0": lambda: op_d(0), "d1": lambda: op_d(1),
                "e0": lambda: op_e(0), "e1": lambda: op_e(1),
                "h8s0": lambda: op_h8s(0), "h8s1": lambda: op_h8s(1),
                "n80": lambda: op_n8(0), "n81": lambda: op_n8(1),
                "hbn0": lambda: op_hbn(0), "hbn1": lambda: op_hbn(1),
                "mf": lambda: nc.vector.tensor_mul(mF[:, :], psN[:, :], r_sb[:, :]),
                "t2f": lambda: nc.vector.tensor_add(t2F[:, :], psG[:, :], mF[:, :]),
                "t20g": lambda: nc.vector.tensor_add(t2_sb[0][:, :], psG[:, HH[0]],
                                                     mF[:, HH[0]]),
                "t21g": lambda: nc.vector.tensor_add(t2_sb[1][:, :], psG[:, HH[1]],
                                                     mF[:, HH[1]]),
                "srp": lambda: nc.scalar.activation(
                    r_ps[:, :], psRZ[:, 0:256], AF.Sigmoid, scale=1.0 / S),
                "mfp": lambda: nc.vector.tensor_mul(mF[:, :], psN[:, :], r_ps[:, :]),
                "thf": lambda: nc.scalar.activation(n_sb[:, :], t2F[:, :], AF.Tanh,
                                                    scale=1.0 / S),
                "zcf": lambda: nc.gpsimd.tensor_scalar(
                    zc_sb[:, :], z_sb[:, :], -SH, SH,
                    op0=mybir.AluOpType.mult, op1=mybir.AluOpType.add),
                "uf": lambda: nc.gpsimd.tensor_mul(u_sb[:, :], z_sb[:, :], h_bf[:, :]),
                "vf": lambda: nc.vector.tensor_mul(vF[:, :], n_sb[:, :], zc_sb[:, :]),
                "h8f": lambda: nc.vector.tensor_add(hn_8[:, :], u_sb[:, :], vF[:, :]),
                "hbf": lambda: nc.gpsimd.tensor_add(hn_bf[:, :], u_sb[:, :], vF[:, :]),
                "hbfd": lambda: nc.vector.tensor_add(hn_bf[:, :], u_sb[:, :], vF[:, :]),
                "df": lambda: nc.vector.scalar_tensor_tensor(
                    dF[:, :], n_sb[:, :], -SH, h_bf[:, :],
                    op0=mybir.AluOpType.mult, op1=mybir.AluOpType.add),
                "ef": lambda: nc.vector.tensor_mul(eF[:, :], z_sb[:, :], dF[:, :]),
                "h8sf": lambda: nc.vector.scalar_tensor_tensor(
                    hn_8[:, :], n_sb[:, :], SH, eF[:, :],
                    op0=mybir.AluOpType.mult, op1=mybir.AluOpType.add),
                "n8f": lambda: nc.gpsimd.tensor_scalar_mul(n8F[:, :], n_sb[:, :], SH),
                "hbnf": lambda: nc.gpsimd.tensor_add(hn_bf[:, :], n8F[:, :], eF[:, :]),
                "zcfd": lambda: nc.vector.tensor_scalar(
                    zc_sb[:, :], z_sb[:, :], -SH, SH,
                    op0=mybir.AluOpType.mult, op1=mybir.AluOpType.add),
                "ufd": lambda: nc.vector.tensor_mul(u_sb[:, :], z_sb[:, :], h_bf[:, :]),
                "szc": lambda: nc.scalar.activation(
                    zc_sb[:, :], psRZ[:, 256:512], AF.Sigmoid, scale=-1.0 / S),  # unscaled (s5 unused)
                "th0F": lambda: nc.scalar.activation(
                    n_sb[:, HH[0]], t2F[:, HH[0]], AF.Tanh, scale=1.0 / S),
                "th1F": lambda: nc.scalar.activation(
                    n_sb[:, HH[1]], t2F[:, HH[1]], AF.Tanh, scale=1.0 / S),
                "sr": op_sr, "sr0": lambda: op_sr(0), "sr1": lambda: op_sr(1),
                "sz": op_sz, "th0": lambda: op_th(0), "th1": lambda: op_th(1),
                "m0": lambda: op_m(0), "m1": lambda: op_m(1),
                "t20": lambda: op_t2(0), "t21": lambda: op_t2(1),
                "zc0": lambda: op_zc(0, nc.gpsimd), "zc1": lambda: op_zc(1, nc.gpsimd),
                "u0": lambda: op_u(0, nc.gpsimd), "u1": lambda: op_u(1, nc.gpsimd),
                "u0d": lambda: op_u(0, nc.vector), "u1d": lambda: op_u(1, nc.vector),
                "v0": lambda: op_v(0, nc.vector), "v1": lambda: op_v(1, nc.vector),
                "h80": lambda: op_h8(0, nc.vector), "h81": lambda: op_h8(1, nc.vector),
                "h80p": lambda: op_h8(0, nc.gpsimd), "h81p": lambda: op_h8(1, nc.gpsimd),
                "hb0": lambda: op_hb(0, nc.gpsimd), "hb1": lambda: op_hb(1, nc.gpsimd),
                "hb0d": lambda: op_hb(0, nc.vector), "hb1d": lambda: op_hb(1, nc.vector),
            }
            for opname in SCHED:
                OPS[opname]()

            h_bf, h8 = hn_bf, hn_8

        # ---- head: out = relu(h) @ fc_w.T + fc_b (contraction over partitions) ----
        reluh = wpool.tile([128, 4 * B], BF16, tag="reluh", name="reluh")
        nc.scalar.activation(reluh[:, :], h_bf[:, :], AF.Relu)
        ps_out = pngpool.tile([B, 1], F32, tag="g", name="ps_out")
        nc.tensor.matmul(ps_out[:, :], ONES[:, :], FCB[:, :], start=True, stop=False)
        for k in range(4):
            nc.tensor.matmul(
                ps_out[:, :], reluh[:, 64 * k : 64 * k + 64], FCW[:, k : k + 1],
                start=False, stop=(k == 3),
            )
        out_sb = wpool.tile([B, 1], F32, tag="out", name="out_sb")
        nc.vector.tensor_copy(out_sb[:, :], ps_out[:, :])
        nc.sync.dma_start(out=out_d[:, :], in_=out_sb[:, :])


_NC_CACHE: dict[int, bass.Bass] = {}


def _get_nc(T: int = T_FULL) -> bass.Bass:
    if T not in _NC_CACHE:
        _NC_CACHE[T] = build_nc(T)
    return _NC_CACHE[T]


def _prep_shared(w_ih, w_hh, b_ih, b_hh, fc_w, fc_b):
    w_hh = np.asarray(w_hh, np.float32)
    wi = np.asarray(w_ih, np.float32)[:, 0]
    b_ih = np.asarray(b_ih, np.float32)
    b_hh = np.asarray(b_hh, np.float32)
    fc_w = np.asarray(fc_w, np.float32)
    fc_b = np.asarray(fc_b, np.float32)

    W8 = (SW * w_hh).astype(NP_FP8)  # [1536, 512]
    ws = np.zeros((128, 12 * 4 * 128), dtype=NP_FP8)
    for s in range(12):
        g, c = s // 4, s % 4
        blk = W8[512 * g + 128 * c : 512 * g + 128 * (c + 1), :]  # [128 j, 512 k]
        for p in range(2):
            for i in range(2):
                col = ((s * 2 + p) * 2 + i) * 128
                ws[:, col : col + 128] = blk[:, 128 * (2 * p + i) : 128 * (2 * p + i + 1)].T

    aug = np.zeros((2, 16 * 128), dtype=np.float32)
    bsum = b_ih + b_hh
    for s in range(8):  # r,z slices
        g, c = s // 4, s % 4
        rows = slice(512 * g + 128 * c, 512 * g + 128 * (c + 1))
        aug[0, s * 128 : (s + 1) * 128] = S * wi[rows]
        aug[1, s * 128 : (s + 1) * 128] = S * bsum[rows]
    for c in range(4):  # n slices: only b_hh (inside the r* product)
        rows = slice(2 * H + 128 * c, 2 * H + 128 * (c + 1))
        aug[1, (8 + c) * 128 : (9 + c) * 128] = S * b_hh[rows]
        aug[0, (12 + c) * 128 : (13 + c) * 128] = S * wi[rows]
        aug[1, (12 + c) * 128 : (13 + c) * 128] = S * b_ih[rows]
    aug = aug.astype(NP_BF16)

    # head reads the pre-scaled master h~ = 8h, so fold the 1/8 into fc_w
    fcw = np.zeros((128, 4), dtype=np.float32)
    for k in range(4):
        fcw[:, k] = fc_w[0, 128 * k : 128 * (k + 1)] / SH
    fcw = fcw.astype(NP_BF16)
    fcb = fc_b.reshape(1, 1).astype(np.float32)
    return {"ws": ws, "aug": aug, "fcw": fcw, "fcb": fcb}


def _prep_xa(x_core):
    # xa[0, t*64 + j] = x_core[j, t]; xa[1, :] = 1.0
    T = x_core.shape[1]
    xa = np.ones((2, T * B), dtype=np.float32)
    xa[0, :] = x_core.T.reshape(-1)
    return xa.astype(NP_BF16)


def kernel(x, w_ih, w_hh, b_ih, b_hh, fc_w, fc_b, _trace=False, _tmpdir=None):
    x = np.ascontiguousarray(np.asarray(x, dtype=np.float32))
    nc = _get_nc(x.shape[1])
    shared = _prep_shared(w_ih, w_hh, b_ih, b_hh, fc_w, fc_b)
    in_maps = [
        {"xa": _prep_xa(x[c * B : (c + 1) * B]), **shared} for c in range(N_CORES)
    ]
    res = run_bass_kernel_spmd(
        nc, in_maps, list(range(N_CORES)), trace=_trace, tmpdir=_tmpdir
    )
    out = np.concatenate([res.results[c]["out"] for c in range(N_CORES)], axis=0)
    if _trace:
        return out, res
    return out
